# revision 18
# baseline (speedup 1.0000x reference)
"""DiT block kernel for Trainium2, 8-way data parallel (one batch element per core).

v2: fp8-e4m3 + DoubleRow for the big GEMMs (qkv, proj, fc1, fc2), bf16 for the
ada GEMV and attention (scores / attn@v).  Weights are host-quantized to e4m3
with a fixed power-of-two scale S=1024 (values are ~N(0, 0.02) so the range
fits comfortably); the 1/S dequant is folded into the existing PSUM-drain
epilogues (ACT scale= or DVE scalar_tensor_tensor).  Activations feed the PE in
raw e4m3 (no runtime scaling needed: e4m3's relative precision is
scale-invariant and all activation magnitudes fit the +-240 range).

DoubleRow packs the contraction dim in pairs (K=256 per matmul): feature-major
activations (hT / h2T / oTb) are stored as 10 blocks of 128 (1152 padded to
1280 with a zeroed block) so adjacent block pairs form the [Ki,2,*] access
pattern; W_qkv/W_proj/W_fc1 are host-packed as [128, 5, 2, N] with zero row
padding, W_fc2 (contraction 4608 = 18*256 exact) as [128, 18, 2, C].

Other changes vs v1:
- gelu(approx) replaced by x*sigmoid(1.702x) == silu(1.702x)/1.702 with the
  1/1.702 folded into W_fc2 and the 1.702 into b_fc1 host-side: the MLP
  activation lives in the same ACT table set as the ada silu.
- LayerNorm rsqrt via exp(-0.5*ln(var+eps)): Ln/Exp/Square/Identity all live
  in one ACT table set, so a rep does 2 table loads (silu<->exp) instead of 5.
- modulate PSUM drains batched 4 token-tiles wide (one N=512 ACT op per
  feature block instead of 4 N=128 ops).
- attention: single [128,1024] score PSUM per (head, m) -> one N=1024 exp;
  softmax normalization drains on DVE only (no ACT copies); o^T written fp8.

DIT_REPS (env, default 1) replicates the block serially inside one NEFF so a
single dispatch amortizes per-dispatch overhead when benchmarking.
"""

import sys
from contextlib import ExitStack

for _p in ("/opt/trn_rl_repo",):
    if _p not in sys.path:
        sys.path.append(_p)

import numpy as np

import concourse.bass as bass
import concourse.mybir as mybir
import concourse.tile as tile
from concourse import bacc
from concourse.bass_utils import run_bass_kernel_spmd
from concourse.masks import make_identity

F32 = mybir.dt.float32
F32R = mybir.dt.float32r
BF16 = mybir.dt.bfloat16
FP8 = mybir.dt.float8e4
AF = mybir.ActivationFunctionType
ALU = mybir.AluOpType
DR = mybir.MatmulPerfMode.DoubleRow

L, C, H, D, FF = 1024, 1152, 16, 72, 4608
P = 128
LT = L // P  # 8 token tiles
CT = C // P  # 9 feature blocks
KP = 5  # DoubleRow steps over the (zero-padded) 1280-row C contraction
KB = 10  # feature blocks incl. zero pad block
FT = FF // P  # 36 ff blocks
NCH = 3  # fc2 chunks (12 ff blocks = 6 DoubleRow steps each)
EPS = 1e-6
SCL = float(D) ** -0.5
SW = 1024.0  # host-side fp8 weight scale (power of two)
DQ = 1.0 / SW
GF = 1.702  # gelu ~= silu(GF*x)/GF


def _mm(nc, out, lhsT, rhs, start, stop, pm=None):
    nc.tensor.matmul(out, lhsT, rhs, start=start, stop=stop, perf_mode=pm)


def build_nc():
    nc = bacc.Bacc(None, target_bir_lowering=False, debug=False)

    x_in = nc.declare_dram_parameter("x", [L, C], F32, isOutput=False)
    c_in = nc.declare_dram_parameter("c", [C], F32, isOutput=False)
    w_qkv = nc.declare_dram_parameter("W_qkv", [P, KP, 2, 3 * C], FP8, isOutput=False)
    b_qkv = nc.declare_dram_parameter("b_qkv", [3 * C], F32, isOutput=False)
    w_proj = nc.declare_dram_parameter("W_proj", [P, KP, 2, C], FP8, isOutput=False)
    b_proj = nc.declare_dram_parameter("b_proj", [C], F32, isOutput=False)
    w_fc1 = nc.declare_dram_parameter("W_fc1", [P, KP, 2, FF], FP8, isOutput=False)
    b_fc1 = nc.declare_dram_parameter("b_fc1", [FF], F32, isOutput=False)
    w_fc2 = nc.declare_dram_parameter("W_fc2", [P, 18, 2, C], FP8, isOutput=False)
    b_fc2 = nc.declare_dram_parameter("b_fc2", [C], F32, isOutput=False)
    w_ada = nc.declare_dram_parameter("W_ada", [C, 6 * C], BF16, isOutput=False)
    b_ada = nc.declare_dram_parameter("b_ada", [6 * C], F32, isOutput=False)
    out = nc.declare_dram_parameter("out", [L, C], F32, isOutput=True)

    x_r = x_in.rearrange("(i p) c -> i p c", p=P)
    out_r = out.rearrange("(i p) c -> i p c", p=P)
    wada_r = w_ada.rearrange("(k p) n -> p k n", p=P)

    with tile.TileContext(nc) as tc:
        import os as _os
        for _rep in range(int(_os.environ.get("DIT_REPS", "1"))):
            with (
                tc.tile_pool(name=f"dram{_rep}", bufs=1, space="DRAM") as dram,
                tc.tile_pool(name=f"const{_rep}", bufs=1) as const,
                tc.tile_pool(name=f"xp{_rep}", bufs=1) as xp,
                tc.tile_pool(name=f"pv{_rep}", bufs=1) as pvp,
                tc.tile_pool(name=f"hTs{_rep}", bufs=2) as hTs,
                tc.tile_pool(name=f"wq{_rep}", bufs=3) as wq,
                tc.tile_pool(name=f"bcp{_rep}", bufs=2) as bcp,
                tc.tile_pool(name=f"stats{_rep}", bufs=12) as stats,
                tc.tile_pool(name=f"xhp{_rep}", bufs=6) as xhp,
            ):
                qkT_hbm = dram.tile([2 * C, L], BF16)
                oT_hbm = dram.tile([C, L], FP8)
                ada_hbm = dram.tile([1, 6 * C], F32)

                identity = const.tile([P, P], BF16)
                make_identity(nc, identity)
                ones = const.tile([1, P], F32R)
                nc.scalar.activation(ones[:], nc.const_aps.tensor(1.0, (1, P)), AF.Identity)
                adaT = const.tile([P, 54], F32)
                sm1_msa = const.tile([P, CT], F32)
                sm1_mlp = const.tile([P, CT], F32)
                bqkv_fm = const.tile([P, 2 * CT], F32)
                bfc1_fm = const.tile([P, FT], F32)
                cT = const.tile([P, CT], F32)
                siluc = const.tile([P, CT], BF16)
                eps_t = const.tile([P, 1], F32)
                nc.gpsimd.memset(eps_t[:], EPS)

                x_sb = xp.tile([P, LT, C], F32)
                # v_aug and y_sb have disjoint lifetimes -> same slot
                v_aug = pvp.tile([P, LT, H, D + 1], BF16, tag="pv", name="v_aug")

                nc.sync.dma_start(cT[:], c_in.rearrange("(k p) -> p k", p=P))
                nc.scalar.activation(siluc[:], cT[:], AF.Silu)

                def bcast(tc, psum_pool, dst, src_row_ap, nm):
                    # replicate a (1,C) row onto all 128 partitions: ones outer-product
                    row = tc_rows.tile([1, C], F32R, tag="bc_row", name=f"bcrow_{nm}", bufs=1)
                    nc.sync.dma_start(row[:], src_row_ap.bitcast(F32R))
                    for n3 in range(3):
                        ps = psum_pool.tile([P, 384], F32, tag="bc", name=f"bc_ps_{nm}_{n3}")
                        sl = slice(n3 * 384, (n3 + 1) * 384)
                        _mm(nc, ps[:], ones[:], row[:, sl], True, True)
                        nc.vector.tensor_copy(dst[:, sl], ps[:])

                def ada_chunks(psum_pool, wpool, rpool, n_lo, n_hi):
                    # ada[chunk] = silu(c) @ W_ada[:, chunk] + b_ada[chunk]  (bf16 GEMV)
                    for n in range(n_lo, n_hi):
                        c0 = n * 512
                        w = min(512, 6 * C - c0)
                        ps = psum_pool.tile([1, 512], F32, tag="ada", name=f"ada_ps_{n}")
                        wt = wpool.tile([P, CT, 512], BF16, tag="wada", name=f"wada_{n}")
                        nc.sync.dma_start(wt[:, :, :w], wada_r[:, :, c0 : c0 + w])
                        for k in range(CT):
                            _mm(nc, ps[:, :w], siluc[:, k : k + 1], wt[:, k, :w], k == 0, k == CT - 1)
                        brow = rpool.tile([1, 512], F32, tag="brow", name=f"bada_{n}")
                        nc.sync.dma_start(
                            brow[:, :w],
                            b_ada[c0 : c0 + w].rearrange("(o n) -> o n", o=1),
                        )
                        row = rpool.tile([1, 512], F32, tag="ada_row", name=f"ada_row_{n}")
                        nc.vector.tensor_add(row[:, :w], ps[:, :w], brow[:, :w])
                        nc.sync.dma_start(ada_hbm[0:1, c0 : c0 + w], row[:, :w])

                def layernorm_xhat(i, xhat):
                    xi = x_sb[:, i, :]
                    # mean/var in one DVE pass (3x 384-chunks, then aggregate)
                    bs = stats.tile([P, 3, 6], F32, tag="bs", name=f"bs_{i}", bufs=2)
                    for kb in range(3):
                        nc.vector.bn_stats(bs[:, kb, :], xi[:, kb * 384 : (kb + 1) * 384])
                    mv = stats.tile([P, 2], F32, tag="mv", name=f"mv_{i}", bufs=2)
                    nc.vector.bn_aggr(mv[:], bs[:])
                    mu, var = mv[:, 0:1], mv[:, 1:2]
                    # rsqrt via exp(-0.5*ln(var+eps)): stays in the exp table set
                    lnv = stats.tile([P, 1], F32, tag="st", name=f"lnv_{i}")
                    nc.scalar.activation(lnv[:], var, AF.Ln, bias=eps_t[:])
                    rs = stats.tile([P, 1], F32, tag="st", name=f"rs_{i}")
                    nc.scalar.activation(rs[:], lnv[:], AF.Exp, scale=-0.5)
                    nmr = stats.tile([P, 1], F32, tag="st", name=f"nmr_{i}")
                    nc.vector.scalar_tensor_tensor(
                        nmr[:], mu, -1.0, rs[:], ALU.mult, ALU.mult
                    )
                    nc.vector.tensor_scalar(xhat[:], xi, rs[:], nmr[:], ALU.mult, ALU.add)

                def modulate_transpose4(trps, i0, xhats, dst, shift_col0, sm1):
                    # 4 token tiles at once: transpose into one [128,512] PSUM,
                    # single N=512 modulate drain per feature block
                    nb = len(xhats)
                    for j in range(CT):
                        tp = trps.tile([P, 4 * P], BF16, tag="tr", name=f"tr_{i0}_{j}")
                        for di in range(nb):
                            nc.tensor.transpose(
                                tp[:, di * P : (di + 1) * P],
                                xhats[di][:, j * P : (j + 1) * P],
                                identity[:],
                            )
                        nc.scalar.activation(
                            dst[:, j, i0 * P : (i0 + nb) * P],
                            tp[:, : nb * P],
                            AF.Identity,
                            bias=adaT[:, shift_col0 + j : shift_col0 + j + 1],
                            scale=sm1[:, j : j + 1],
                        )

                # ======== phase A: ada (msa half) + LN1 -> hT (fp8) ========
                hT = hTs.tile([P, KB, L], FP8, tag="hT", name="hT1")
                nc.gpsimd.memset(hT[:, KB - 1, :], 0.0)
                with (
                    tc.tile_pool(name=f"rows{_rep}", bufs=2) as tc_rows,
                    tc.tile_pool(name=f"wada_p{_rep}", bufs=2) as wada_p,
                    tc.tile_pool(name=f"ada_ps{_rep}", bufs=2, space="PSUM") as ada_ps,
                    tc.tile_pool(name=f"tr_ps1{_rep}", bufs=2, space="PSUM") as tr_ps1,
                ):
                    ada_chunks(ada_ps, wada_p, tc_rows, 0, 5)
                    nc.sync.dma_start(bqkv_fm[:], b_qkv[0 : 2 * C].rearrange("(j p) -> p j", p=P))
                    nc.sync.dma_start(bfc1_fm[:], b_fc1.rearrange("(j p) -> p j", p=P))
                    for i in range(LT):
                        nc.sync.dma_start(x_sb[:, i, :], x_r[i])
                    nc.sync.dma_start(
                        adaT[:, 0:20], ada_hbm[0, 0:2560].rearrange("(g p) -> p g", p=P)
                    )
                    nc.vector.tensor_scalar_add(sm1_msa[:], adaT[:, 9:18], 1.0)
                    xh4 = []
                    for i in range(LT):
                        xh = xhp.tile([P, C], BF16, tag="xh", name=f"xh1_{i}")
                        layernorm_xhat(i, xh)
                        xh4.append(xh)
                        if i % 4 == 3:
                            modulate_transpose4(tr_ps1, i - 3, xh4, hT, 0, sm1_msa)
                            xh4 = []

                # ======== phase B: v = h @ W_v (token-major, fp8 DoubleRow) ========
                nc.scalar.activation(
                    v_aug[:, :, :, D : D + 1],
                    nc.const_aps.tensor(1.0, (P, LT, H, 1)),
                    AF.Identity,
                )
                with tc.tile_pool(name=f"rows2{_rep}", bufs=2) as tc_rows:
                    with tc.tile_pool(name=f"bv_ps{_rep}", bufs=2, space="PSUM") as bv_ps:
                        bv_bc = bcp.tile([P, C], F32, tag="bc", name="bv_bc")
                        bcast(tc, bv_ps, bv_bc, b_qkv[2 * C : 3 * C].rearrange("(o n) -> o n", o=1), "bv")
                    with tc.tile_pool(name=f"v_ps{_rep}", bufs=4, space="PSUM") as v_ps:
                        for n4 in range(4):
                            wv = wq.tile([P, KP, 2, 288], FP8, tag="wv", name=f"wv_{n4}")
                            nc.sync.dma_start(
                                wv[:], w_qkv[:, :, :, 2 * C + n4 * 288 : 2 * C + (n4 + 1) * 288]
                            )
                            for i in range(LT):
                                ps = v_ps.tile([P, 288], F32, tag="vps", name=f"v_ps_{n4}_{i}")
                                for k in range(KP):
                                    _mm(
                                        nc,
                                        ps[:],
                                        hT[:, 2 * k : 2 * k + 2, i * P : (i + 1) * P],
                                        wv[:, k, :, :],
                                        k == 0,
                                        k == KP - 1,
                                        pm=DR,
                                    )
                                nc.vector.scalar_tensor_tensor(
                                    v_aug[:, i, n4 * 4 : (n4 + 1) * 4, 0:D],
                                    ps[:].rearrange("p (h d) -> p h d", d=D),
                                    DQ,
                                    bv_bc[:, n4 * 288 : (n4 + 1) * 288].rearrange(
                                        "p (h d) -> p h d", d=D
                                    ),
                                    ALU.mult,
                                    ALU.add,
                                )

                # ======== phase C: q^T,k^T feature-major (fp8 DR) -> DRAM ========
                # ada (mlp half) interleaved so its W stream + GEMVs overlap
                # the qk matmuls instead of delaying attention start
                with (
                    tc.tile_pool(name=f"wqk_p{_rep}", bufs=2) as wqk_p,
                    tc.tile_pool(name=f"qkb_p{_rep}", bufs=2) as qkb_p,
                    tc.tile_pool(name=f"rows2b{_rep}", bufs=2) as tc_rows,
                    tc.tile_pool(name=f"wada2{_rep}", bufs=2) as wada2_p,
                    tc.tile_pool(name=f"qk_ps{_rep}", bufs=2, space="PSUM") as qk_ps,
                    tc.tile_pool(name=f"ada_ps2{_rep}", bufs=2, space="PSUM") as ada_ps2,
                ):
                    wqk_tiles = {}
                    for jb in range(2 * CT):
                        if jb % 2 == 0 and jb // 2 < 9:
                            ada_chunks(ada_ps2, wada2_p, tc_rows, 5 + jb // 2, 6 + jb // 2)
                        g, r = divmod(jb * P, 512)
                        if r == 0:
                            gw = min(512, 2 * C - g * 512)
                            gt = wqk_p.tile([P, KP, 2, 512], FP8, tag="wqk", name=f"wqkg_{g}")
                            nc.sync.dma_start(
                                gt[:, :, :, :gw], w_qkv[:, :, :, g * 512 : g * 512 + gw]
                            )
                            wqk_tiles[g] = gt
                        wt = wqk_tiles[g]
                        ps = qk_ps.tile([P, L], F32, tag="qkps", name=f"qk_ps_{jb}")
                        for k in range(KP):
                            for n2 in range(2):
                                _mm(
                                    nc,
                                    ps[:, n2 * 512 : (n2 + 1) * 512],
                                    wt[:, k, :, r : r + P],
                                    hT[:, 2 * k : 2 * k + 2, n2 * 512 : (n2 + 1) * 512],
                                    k == 0,
                                    k == KP - 1,
                                    pm=DR,
                                )
                        qkb = qkb_p.tile([P, L], BF16, tag="qkb", name=f"qkb_{jb}")
                        nc.vector.tensor_scalar(
                            qkb[:], ps[:], DQ, bqkv_fm[:, jb : jb + 1],
                            ALU.mult, ALU.add,
                        )
                        nc.sync.dma_start(qkT_hbm[jb * P : (jb + 1) * P, :], qkb[:])

                # hoist the first head's q/k loads ahead of the ada-mlp stream
                qk0 = [const.tile([D, L], BF16, tag=f"qk0_{t}", name=f"qk0_{t}") for t in range(4)]
                for h in (0, 1):
                    nc.sync.dma_start(qk0[2 * h][:], qkT_hbm[h * D : (h + 1) * D, :])
                    nc.sync.dma_start(qk0[2 * h + 1][:], qkT_hbm[C + h * D : C + (h + 1) * D, :])

                # ======== remaining adaT ========
                nc.sync.dma_start(
                    adaT[:, 20:54], ada_hbm[0, 2560 : 6 * C].rearrange("(g p) -> p g", p=P)
                )
                nc.vector.tensor_scalar_add(sm1_mlp[:], adaT[:, 36:45], 1.0)

                # ======== phase D: attention, head at a time ========
                with (
                    tc.tile_pool(name=f"qkh{_rep}", bufs=4) as qkh,
                    tc.tile_pool(name=f"eTp{_rep}", bufs=3) as eTp,
                    tc.tile_pool(name=f"zp{_rep}", bufs=4) as zp,
                    tc.tile_pool(name=f"op{_rep}", bufs=2) as op_pool,
                    tc.tile_pool(name=f"s_ps{_rep}", bufs=2, space="PSUM") as s_ps,
                    tc.tile_pool(name=f"o_ps{_rep}", bufs=2, space="PSUM") as o_ps,
                ):
                    pend = []

                    def attn_finish():
                        h, o_sb, rz = pend.pop()
                        # broadcast 1/z to D partitions (PE) and scale o on DVE
                        psr = o_ps.tile([D, L], F32, tag="ops", name=f"psr_{h}")
                        for n2 in range(2):
                            sl = slice(n2 * 512, (n2 + 1) * 512)
                            _mm(nc, psr[:, sl], ones[:, 0:D], rz[:, sl], True, True)
                        oT = op_pool.tile([D, L], FP8, tag="oT", name=f"oT_{h}")
                        nc.vector.tensor_mul(oT[:], o_sb[0:D, :], psr[:])
                        nc.sync.dma_start(oT_hbm[h * D : (h + 1) * D, :], oT[:])

                    for h in range(H):
                        if h < 2:
                            qT, kT = qk0[2 * h][:], qk0[2 * h + 1][:]
                        else:
                            qTt = qkh.tile([D, L], BF16, tag="qkh", name=f"qT_{h}")
                            nc.sync.dma_start(qTt[:], qkT_hbm[h * D : (h + 1) * D, :])
                            kTt = qkh.tile([D, L], BF16, tag="qkh", name=f"kT_{h}")
                            nc.sync.dma_start(kTt[:], qkT_hbm[C + h * D : C + (h + 1) * D, :])
                            qT, kT = qTt[:], kTt[:]
                        pso = o_ps.tile([D + 1, L], F32, tag="ops", name=f"o_ps_{h}")
                        for m in range(LT):
                            sps = s_ps.tile([P, L], F32, tag="sps", name=f"s_ps_{h}_{m}")
                            for n2 in range(2):
                                _mm(
                                    nc,
                                    sps[:, n2 * 512 : (n2 + 1) * 512],
                                    kT[:, m * P : (m + 1) * P],
                                    qT[:, n2 * 512 : (n2 + 1) * 512],
                                    True,
                                    True,
                                )
                            eT = eTp.tile([P, L], BF16, tag="eT", name=f"eT_{h}_{m}")
                            nc.scalar.activation(eT[:], sps[:], AF.Exp, scale=SCL)
                            for n2 in range(2):
                                _mm(
                                    nc,
                                    pso[:, n2 * 512 : (n2 + 1) * 512],
                                    v_aug[:, m, h, :],
                                    eT[:, n2 * 512 : (n2 + 1) * 512],
                                    m == 0,
                                    m == LT - 1,
                                )
                            if m == 2 and pend:
                                attn_finish()
                        # drain o + softmax denominator (DVE only)
                        o_sb = zp.tile([D + 1, L], F32, tag="osb", name=f"osb_{h}", bufs=2)
                        nc.vector.tensor_copy(o_sb[:], pso[:])
                        z_row = zp.tile([1, L], F32, tag="z", name=f"z_{h}", bufs=4)
                        nc.sync.dma_start(z_row[:], o_sb[D : D + 1, :])
                        rz = zp.tile([1, L], F32R, tag="z", name=f"rz_{h}", bufs=4)
                        with nc.allow_low_precision(reason="f32r is fp32-width"):
                            nc.vector.reciprocal(rz[:], z_row[:].bitcast(F32R))
                        pend.append((h, o_sb, rz[:]))
                    attn_finish()

                # ======== phase E: proj (fp8 DR) + gated residual; LN2 folded ========
                oTb = hTs.tile([P, KB, L], FP8, tag="hT", name="oTb")
                nc.gpsimd.memset(oTb[:, KB - 1, :], 0.0)
                nc.sync.dma_start(
                    oTb[:, 0:CT, :], oT_hbm[:].rearrange("(k p) l -> p k l", p=P)
                )
                wpj = wq.tile([P, KP, 2, C], FP8, tag="wpj", name="wpj", bufs=1)
                nc.sync.dma_start(wpj[:], w_proj[:])
                h2T = hTs.tile([P, KB, L], FP8, tag="hT", name="h2T")
                nc.gpsimd.memset(h2T[:, KB - 1, :], 0.0)
                xh2s = []
                with (
                    tc.tile_pool(name=f"rows3{_rep}", bufs=2) as tc_rows,
                    tc.tile_pool(name=f"resp{_rep}", bufs=3) as resp,
                    tc.tile_pool(name=f"bc_ps3{_rep}", bufs=2, space="PSUM") as bc_ps3,
                    tc.tile_pool(name=f"pj_ps{_rep}", bufs=4, space="PSUM") as pj_ps,
                    tc.tile_pool(name=f"tr_ps2{_rep}", bufs=2, space="PSUM") as tr_ps2,
                ):
                    gmsa_bc = bcp.tile([P, C], F32, tag="bc", name="gmsa_bc")
                    bcast(tc, bc_ps3, gmsa_bc, ada_hbm[0:1, 2 * C : 3 * C], "gmsa")
                    bproj_bc = bcp.tile([P, C], F32, tag="bc", name="bproj_bc")
                    bcast(tc, bc_ps3, bproj_bc, b_proj.rearrange("(o n) -> o n", o=1), "bproj")
                    for i in range(LT):
                        for c0, cw in ((0, 512), (512, 512), (1024, 128)):
                            ps = pj_ps.tile([P, 512], F32, tag="pjps", name=f"pj_ps_{i}_{c0}")
                            for k in range(KP):
                                _mm(
                                    nc,
                                    ps[:, :cw],
                                    oTb[:, 2 * k : 2 * k + 2, i * P : (i + 1) * P],
                                    wpj[:, k, :, c0 : c0 + cw],
                                    k == 0,
                                    k == KP - 1,
                                    pm=DR,
                                )
                            sl = slice(c0, c0 + cw)
                            t = resp.tile([P, 512], F32, tag="res", name=f"res_{i}_{c0}")
                            nc.vector.scalar_tensor_tensor(
                                t[:, :cw], ps[:, :cw], DQ, bproj_bc[:, sl], ALU.mult, ALU.add
                            )
                            nc.vector.tensor_mul(t[:, :cw], t[:, :cw], gmsa_bc[:, sl])
                            nc.vector.tensor_add(x_sb[:, i, sl], x_sb[:, i, sl], t[:, :cw])
                        # LN2 stats/xhat for tile i overlap remaining proj work
                        xh = xhp.tile([P, C], BF16, tag="xh", name=f"xh2_{i}")
                        layernorm_xhat(i, xh)
                        xh2s.append(xh)
                        if i % 4 == 3:
                            modulate_transpose4(tr_ps2, i - 3, xh2s, h2T, 27, sm1_mlp)
                            xh2s = []

                # ======== phase G: MLP (fp8 DR) + final residual ========
                y_sb = pvp.tile([P, LT, C], F32, tag="pv", name="y_sb")
                JPC = FT // NCH  # 12 ff blocks per chunk
                with (
                    tc.tile_pool(name=f"rows4{_rep}", bufs=2) as tc_rows,
                    tc.tile_pool(name=f"wf1_p{_rep}", bufs=3) as wf1_p,
                    tc.tile_pool(name=f"uTs{_rep}", bufs=2) as uTs,
                    tc.tile_pool(name=f"resp2{_rep}", bufs=2) as resp2,
                ):
                    with tc.tile_pool(name=f"bc_ps4{_rep}", bufs=2, space="PSUM") as bc_ps4:
                        bfc2_bc = bcp.tile([P, C], F32, tag="bc", name="bfc2_bc")
                        bcast(tc, bc_ps4, bfc2_bc, b_fc2.rearrange("(o n) -> o n", o=1), "bfc2")
                        gmlp_bc = bcp.tile([P, C], F32, tag="bc", name="gmlp_bc")
                        bcast(tc, bc_ps4, gmlp_bc, ada_hbm[0:1, 5 * C : 6 * C], "gmlp")
                    mlp_ps = ExitStack()
                    f1_ps = mlp_ps.enter_context(tc.tile_pool(name=f"f1_ps{_rep}", bufs=2, space="PSUM"))
                    f2_ps = mlp_ps.enter_context(tc.tile_pool(name=f"f2_ps{_rep}", bufs=4, space="PSUM"))
                    for ch in range(NCH):
                        uT = uTs.tile([P, JPC, L], FP8, tag="uT", name=f"uT_{ch}")
                        for jj in range(JPC):
                            j = ch * JPC + jj
                            if j % 2 == 0:
                                wt4 = wf1_p.tile([P, KP, 2, 2 * P], FP8, tag="wf1", name=f"wfc1_{j}")
                                nc.sync.dma_start(wt4[:], w_fc1[:, :, :, j * P : (j + 2) * P])
                            jw = (j % 2) * P
                            ps = f1_ps.tile([P, L], F32, tag="f1ps", name=f"f1_ps_{j}")
                            for k in range(KP):
                                for n2 in range(2):
                                    _mm(
                                        nc,
                                        ps[:, n2 * 512 : (n2 + 1) * 512],
                                        wt4[:, k, :, jw : jw + P],
                                        h2T[:, 2 * k : 2 * k + 2, n2 * 512 : (n2 + 1) * 512],
                                        k == 0,
                                        k == KP - 1,
                                        pm=DR,
                                    )
                            # gelu(x) ~= silu(GF*x)/GF; the 1/GF lives in W_fc2,
                            # the GF in the prescaled b_fc1; DQ dequants the psum
                            nc.scalar.activation(
                                uT[:, jj, :],
                                ps[:],
                                AF.Silu,
                                bias=bfc1_fm[:, j : j + 1],
                                scale=GF * DQ,
                            )
                        for c0, cw in ((0, 512), (512, 512), (1024, 128)):
                            wf2 = wq.tile([P, JPC // 2, 2, 512], FP8, tag="wv", name=f"wfc2_{ch}_{c0}")
                            nc.sync.dma_start(
                                wf2[:, :, :, :cw],
                                w_fc2[:, ch * (JPC // 2) : (ch + 1) * (JPC // 2), :, c0 : c0 + cw],
                            )
                            sl = slice(c0, c0 + cw)
                            for i in range(LT):
                                ps = f2_ps.tile([P, 512], F32, tag="f2ps", name=f"f2_ps_{ch}_{c0}_{i}")
                                for kk in range(JPC // 2):
                                    _mm(
                                        nc,
                                        ps[:, :cw],
                                        uT[:, 2 * kk : 2 * kk + 2, i * P : (i + 1) * P],
                                        wf2[:, kk, :, :cw],
                                        kk == 0,
                                        kk == JPC // 2 - 1,
                                        pm=DR,
                                    )
                                if ch == 0:
                                    nc.vector.scalar_tensor_tensor(
                                        y_sb[:, i, sl], ps[:, :cw], DQ, bfc2_bc[:, sl],
                                        ALU.mult, ALU.add,
                                    )
                                elif ch < NCH - 1:
                                    nc.vector.scalar_tensor_tensor(
                                        y_sb[:, i, sl], ps[:, :cw], DQ, y_sb[:, i, sl],
                                        ALU.mult, ALU.add,
                                    )
                                else:
                                    t = resp2.tile([P, 512], F32, tag="res2", name=f"fres_{i}_{c0}")
                                    nc.vector.scalar_tensor_tensor(
                                        t[:, :cw], ps[:, :cw], DQ, y_sb[:, i, sl],
                                        ALU.mult, ALU.add,
                                    )
                                    nc.vector.tensor_mul(t[:, :cw], t[:, :cw], gmlp_bc[:, sl])
                                    nc.vector.tensor_add(
                                        x_sb[:, i, sl], x_sb[:, i, sl], t[:, :cw]
                                    )
                                    if c0 == 1024:
                                        nc.sync.dma_start(out_r[i], x_sb[:, i, :])

                    mlp_ps.close()

    nc.compile()
    return nc


_NC_CACHE = {}


def get_nc():
    if "nc" not in _NC_CACHE:
        _NC_CACHE["nc"] = build_nc()
    return _NC_CACHE["nc"]


def _pack_w(W, scale=SW):
    """[Cin, N] fp32 -> [128, ceil(Cin/256), 2, N] e4m3 with zero row pad."""
    fp8 = mybir.dt.np(FP8)
    Cin, N = W.shape
    kp = -(-Cin // 256)
    Wp = np.zeros((kp * 256, N), np.float32)
    Wp[:Cin] = np.clip(W * scale, -240.0, 240.0)
    return np.ascontiguousarray(
        Wp.reshape(kp, 2, P, N).transpose(2, 0, 1, 3).astype(fp8)
    )


def make_in_maps(inputs):
    import ml_dtypes

    B = inputs["x"].shape[0]
    f32 = lambda k: np.asarray(inputs[k], dtype=np.float32)
    shared = {
        "W_qkv": _pack_w(f32("W_qkv")),
        "W_proj": _pack_w(f32("W_proj")),
        "W_fc1": _pack_w(f32("W_fc1")),
        "W_fc2": _pack_w(f32("W_fc2") / GF),
        "W_ada": np.ascontiguousarray(f32("W_ada").astype(ml_dtypes.bfloat16)),
        "b_qkv": np.ascontiguousarray(f32("b_qkv")),
        "b_proj": np.ascontiguousarray(f32("b_proj")),
        "b_fc1": np.ascontiguousarray(f32("b_fc1") * GF),
        "b_fc2": np.ascontiguousarray(f32("b_fc2")),
        "b_ada": np.ascontiguousarray(f32("b_ada")),
    }
    in_maps = []
    for i in range(B):
        m = dict(shared)
        m["x"] = np.ascontiguousarray(np.asarray(inputs["x"][i], dtype=np.float32))
        m["c"] = np.ascontiguousarray(
            np.asarray(inputs["c"][i], dtype=np.float32).reshape(C)
        )
        in_maps.append(m)
    return in_maps


def kernel(**inputs):
    nc = get_nc()
    in_maps = make_in_maps(inputs)
    res = run_bass_kernel_spmd(nc, in_maps, list(range(len(in_maps))))
    return np.stack([r["out"] for r in res.results]).astype(np.float32)


# revision 19
# speedup vs baseline: 1.1378x; 1.1378x over previous
"""DiT block kernel for Trainium2, 8-way data parallel (one batch element per core).

v2: fp8-e4m3 + DoubleRow for the big GEMMs (qkv, proj, fc1, fc2), bf16 for the
ada GEMV and attention (scores / attn@v).  Weights are host-quantized to e4m3
with a fixed power-of-two scale S=1024 (values are ~N(0, 0.02) so the range
fits comfortably); the 1/S dequant is folded into the existing PSUM-drain
epilogues (ACT scale= or DVE scalar_tensor_tensor).  Activations feed the PE in
raw e4m3 (no runtime scaling needed: e4m3's relative precision is
scale-invariant and all activation magnitudes fit the +-240 range).

DoubleRow packs the contraction dim in pairs (K=256 per matmul): feature-major
activations (hT / h2T / oTb) are stored as 10 blocks of 128 (1152 padded to
1280 with a zeroed block) so adjacent block pairs form the [Ki,2,*] access
pattern; W_qkv/W_proj/W_fc1 are host-packed as [128, 5, 2, N] with zero row
padding, W_fc2 (contraction 4608 = 18*256 exact) as [128, 18, 2, C].

Other changes vs v1:
- gelu(approx) replaced by x*sigmoid(1.702x) == silu(1.702x)/1.702 with the
  1/1.702 folded into W_fc2 and the 1.702 into b_fc1 host-side: the MLP
  activation lives in the same ACT table set as the ada silu.
- LayerNorm rsqrt via exp(-0.5*ln(var+eps)): Ln/Exp/Square/Identity all live
  in one ACT table set, so a rep does 2 table loads (silu<->exp) instead of 5.
- modulate PSUM drains batched 4 token-tiles wide (one N=512 ACT op per
  feature block instead of 4 N=128 ops).
- attention: single [128,1024] score PSUM per (head, m) -> one N=1024 exp;
  softmax normalization drains on DVE only (no ACT copies); o^T written fp8.

DIT_REPS (env, default 1) replicates the block serially inside one NEFF so a
single dispatch amortizes per-dispatch overhead when benchmarking.
"""

import sys
from contextlib import ExitStack

for _p in ("/opt/trn_rl_repo",):
    if _p not in sys.path:
        sys.path.append(_p)

import numpy as np

import concourse.bass as bass
import concourse.mybir as mybir
import concourse.tile as tile
from concourse import bacc
from concourse.bass_utils import run_bass_kernel_spmd
from concourse.masks import make_identity

F32 = mybir.dt.float32
F32R = mybir.dt.float32r
BF16 = mybir.dt.bfloat16
FP8 = mybir.dt.float8e4
AF = mybir.ActivationFunctionType
ALU = mybir.AluOpType
DR = mybir.MatmulPerfMode.DoubleRow

L, C, H, D, FF = 1024, 1152, 16, 72, 4608
P = 128
LT = L // P  # 8 token tiles
CT = C // P  # 9 feature blocks
KP = 5  # DoubleRow steps over the (zero-padded) 1280-row C contraction
KB = 10  # feature blocks incl. zero pad block
FT = FF // P  # 36 ff blocks
NCH = 3  # fc2 chunks (12 ff blocks = 6 DoubleRow steps each)
EPS = 1e-6
SCL = float(D) ** -0.5
SW = 1024.0  # host-side fp8 weight scale (power of two)
DQ = 1.0 / SW
GF = 1.702  # gelu ~= silu(GF*x)/GF


def _mm(nc, out, lhsT, rhs, start, stop, pm=None):
    nc.tensor.matmul(out, lhsT, rhs, start=start, stop=stop, perf_mode=pm)


def build_nc():
    nc = bacc.Bacc(None, target_bir_lowering=False, debug=False)

    x_in = nc.declare_dram_parameter("x", [L, C], F32, isOutput=False)
    c_in = nc.declare_dram_parameter("c", [C], F32, isOutput=False)
    w_qkv = nc.declare_dram_parameter("W_qkv", [P, KP, 2, 3 * C], FP8, isOutput=False)
    b_qkv = nc.declare_dram_parameter("b_qkv", [3 * C], F32, isOutput=False)
    w_proj = nc.declare_dram_parameter("W_proj", [P, KP, 2, C], FP8, isOutput=False)
    b_proj = nc.declare_dram_parameter("b_proj", [C], F32, isOutput=False)
    w_fc1 = nc.declare_dram_parameter("W_fc1", [P, KP, 2, FF], FP8, isOutput=False)
    b_fc1 = nc.declare_dram_parameter("b_fc1", [FF], F32, isOutput=False)
    w_fc2 = nc.declare_dram_parameter("W_fc2", [P, 18, 2, C], FP8, isOutput=False)
    b_fc2 = nc.declare_dram_parameter("b_fc2", [C], F32, isOutput=False)
    w_ada = nc.declare_dram_parameter("W_ada", [C, 6 * C], BF16, isOutput=False)
    b_ada = nc.declare_dram_parameter("b_ada", [6 * C], F32, isOutput=False)
    out = nc.declare_dram_parameter("out", [L, C], F32, isOutput=True)

    x_r = x_in.rearrange("(i p) c -> i p c", p=P)
    out_r = out.rearrange("(i p) c -> i p c", p=P)
    wada_r = w_ada.rearrange("(k p) n -> p k n", p=P)

    with tile.TileContext(nc) as tc:
        import os as _os
        for _rep in range(int(_os.environ.get("DIT_REPS", "1"))):
            with (
                tc.tile_pool(name=f"dram{_rep}", bufs=1, space="DRAM") as dram,
                tc.tile_pool(name=f"const{_rep}", bufs=1) as const,
                tc.tile_pool(name=f"xp{_rep}", bufs=1) as xp,
                tc.tile_pool(name=f"pv{_rep}", bufs=1) as pvp,
                tc.tile_pool(name=f"hTs{_rep}", bufs=2) as hTs,
                tc.tile_pool(name=f"wq{_rep}", bufs=3) as wq,
                tc.tile_pool(name=f"bcp{_rep}", bufs=2) as bcp,
                tc.tile_pool(name=f"stats{_rep}", bufs=12) as stats,
                tc.tile_pool(name=f"xhp{_rep}", bufs=6) as xhp,
            ):
                qkT_hbm = dram.tile([2 * C, L], BF16)
                oT_hbm = dram.tile([C, L], FP8)
                ada_hbm = dram.tile([1, 6 * C], F32)

                identity = const.tile([P, P], BF16)
                make_identity(nc, identity)
                ones = const.tile([1, P], F32R)
                nc.scalar.activation(ones[:], nc.const_aps.tensor(1.0, (1, P)), AF.Identity)
                adaT = const.tile([P, 54], F32)
                sm1_msa = const.tile([P, CT], F32)
                sm1_mlp = const.tile([P, CT], F32)
                bqkv_fm = const.tile([P, 2 * CT], F32)
                bfc1_fm = const.tile([P, FT], F32)
                cT = const.tile([P, CT], F32)
                siluc = const.tile([P, CT], BF16)
                eps_t = const.tile([P, 1], F32)
                nc.gpsimd.memset(eps_t[:], EPS)

                x_sb = xp.tile([P, LT, C], F32)
                # v_aug and y_sb have disjoint lifetimes -> same slot
                v_aug = pvp.tile([P, LT, H, D + 1], BF16, tag="pv", name="v_aug")

                nc.sync.dma_start(cT[:], c_in.rearrange("(k p) -> p k", p=P))
                nc.scalar.activation(siluc[:], cT[:], AF.Silu)

                def bcast(tc, psum_pool, dst, src_row_ap, nm):
                    # replicate a (1,C) row onto all 128 partitions: ones outer-product
                    row = tc_rows.tile([1, C], F32R, tag="bc_row", name=f"bcrow_{nm}", bufs=1)
                    nc.sync.dma_start(row[:], src_row_ap.bitcast(F32R))
                    for n3 in range(3):
                        ps = psum_pool.tile([P, 384], F32, tag="bc", name=f"bc_ps_{nm}_{n3}")
                        sl = slice(n3 * 384, (n3 + 1) * 384)
                        _mm(nc, ps[:], ones[:], row[:, sl], True, True)
                        nc.vector.tensor_copy(dst[:, sl], ps[:])

                def ada_chunks(psum_pool, wpool, rpool, n_lo, n_hi):
                    # ada[chunk] = silu(c) @ W_ada[:, chunk] + b_ada[chunk]  (bf16 GEMV)
                    for n in range(n_lo, n_hi):
                        c0 = n * 512
                        w = min(512, 6 * C - c0)
                        ps = psum_pool.tile([1, 512], F32, tag="ada", name=f"ada_ps_{n}")
                        wt = wpool.tile([P, CT, 512], BF16, tag="wada", name=f"wada_{n}")
                        nc.sync.dma_start(wt[:, :, :w], wada_r[:, :, c0 : c0 + w])
                        for k in range(CT):
                            _mm(nc, ps[:, :w], siluc[:, k : k + 1], wt[:, k, :w], k == 0, k == CT - 1)
                        brow = rpool.tile([1, 512], F32, tag="brow", name=f"bada_{n}")
                        nc.sync.dma_start(
                            brow[:, :w],
                            b_ada[c0 : c0 + w].rearrange("(o n) -> o n", o=1),
                        )
                        row = rpool.tile([1, 512], F32, tag="ada_row", name=f"ada_row_{n}")
                        nc.vector.tensor_add(row[:, :w], ps[:, :w], brow[:, :w])
                        nc.sync.dma_start(ada_hbm[0:1, c0 : c0 + w], row[:, :w])

                def layernorm_xhat(i, xhat):
                    xi = x_sb[:, i, :]
                    # mean/var in one DVE pass (3x 384-chunks, then aggregate)
                    bs = stats.tile([P, 3, 6], F32, tag="bs", name=f"bs_{i}", bufs=2)
                    for kb in range(3):
                        nc.vector.bn_stats(bs[:, kb, :], xi[:, kb * 384 : (kb + 1) * 384])
                    mv = stats.tile([P, 2], F32, tag="mv", name=f"mv_{i}", bufs=2)
                    nc.vector.bn_aggr(mv[:], bs[:])
                    mu, var = mv[:, 0:1], mv[:, 1:2]
                    # rsqrt via exp(-0.5*ln(var+eps)): stays in the exp table set
                    lnv = stats.tile([P, 1], F32, tag="st", name=f"lnv_{i}")
                    nc.scalar.activation(lnv[:], var, AF.Ln, bias=eps_t[:])
                    rs = stats.tile([P, 1], F32, tag="st", name=f"rs_{i}")
                    nc.scalar.activation(rs[:], lnv[:], AF.Exp, scale=-0.5)
                    nmr = stats.tile([P, 1], F32, tag="st", name=f"nmr_{i}")
                    nc.vector.scalar_tensor_tensor(
                        nmr[:], mu, -1.0, rs[:], ALU.mult, ALU.mult
                    )
                    nc.scalar.activation(xhat[:], xi, AF.Identity, bias=nmr[:], scale=rs[:])

                def modulate_transpose4(trps, i0, xhats, dst, shift_col0, sm1):
                    # 4 token tiles at once: transpose into one [128,512] PSUM,
                    # single N=512 modulate drain per feature block
                    nb = len(xhats)
                    for j in range(CT):
                        tp = trps.tile([P, 4 * P], BF16, tag="tr", name=f"tr_{i0}_{j}")
                        for di in range(nb):
                            nc.tensor.transpose(
                                tp[:, di * P : (di + 1) * P],
                                xhats[di][:, j * P : (j + 1) * P],
                                identity[:],
                            )
                        nc.scalar.activation(
                            dst[:, j, i0 * P : (i0 + nb) * P],
                            tp[:, : nb * P],
                            AF.Identity,
                            bias=adaT[:, shift_col0 + j : shift_col0 + j + 1],
                            scale=sm1[:, j : j + 1],
                        )

                # ======== phase A: ada (msa half) + LN1 -> hT (fp8) ========
                hT = hTs.tile([P, KB, L], FP8, tag="hT", name="hT1")
                nc.gpsimd.memset(hT[:, KB - 1, :], 0.0)
                with (
                    tc.tile_pool(name=f"rows{_rep}", bufs=2) as tc_rows,
                    tc.tile_pool(name=f"wada_p{_rep}", bufs=2) as wada_p,
                    tc.tile_pool(name=f"ada_ps{_rep}", bufs=2, space="PSUM") as ada_ps,
                    tc.tile_pool(name=f"tr_ps1{_rep}", bufs=2, space="PSUM") as tr_ps1,
                ):
                    ada_chunks(ada_ps, wada_p, tc_rows, 0, 5)
                    nc.sync.dma_start(bqkv_fm[:], b_qkv[0 : 2 * C].rearrange("(j p) -> p j", p=P))
                    nc.sync.dma_start(bfc1_fm[:], b_fc1.rearrange("(j p) -> p j", p=P))
                    for i in range(LT):
                        nc.sync.dma_start(x_sb[:, i, :], x_r[i])
                    nc.sync.dma_start(
                        adaT[:, 0:20], ada_hbm[0, 0:2560].rearrange("(g p) -> p g", p=P)
                    )
                    nc.vector.tensor_scalar_add(sm1_msa[:], adaT[:, 9:18], 1.0)
                    xh4 = []
                    for i in range(LT):
                        xh = xhp.tile([P, C], BF16, tag="xh", name=f"xh1_{i}")
                        layernorm_xhat(i, xh)
                        xh4.append(xh)
                        if i % 4 == 3:
                            modulate_transpose4(tr_ps1, i - 3, xh4, hT, 0, sm1_msa)
                            xh4 = []

                # ======== phase B: v = h @ W_v (token-major, fp8 DoubleRow) ========
                nc.scalar.activation(
                    v_aug[:, :, :, D : D + 1],
                    nc.const_aps.tensor(1.0, (P, LT, H, 1)),
                    AF.Identity,
                )
                with tc.tile_pool(name=f"rows2{_rep}", bufs=2) as tc_rows:
                    with tc.tile_pool(name=f"bv_ps{_rep}", bufs=2, space="PSUM") as bv_ps:
                        bv_bc = bcp.tile([P, C], F32, tag="bc", name="bv_bc")
                        bcast(tc, bv_ps, bv_bc, b_qkv[2 * C : 3 * C].rearrange("(o n) -> o n", o=1), "bv")
                    with tc.tile_pool(name=f"v_ps{_rep}", bufs=4, space="PSUM") as v_ps:
                        for n4 in range(4):
                            wv = wq.tile([P, KP, 2, 288], FP8, tag="wv", name=f"wv_{n4}")
                            nc.sync.dma_start(
                                wv[:], w_qkv[:, :, :, 2 * C + n4 * 288 : 2 * C + (n4 + 1) * 288]
                            )
                            for i in range(LT):
                                ps = v_ps.tile([P, 288], F32, tag="vps", name=f"v_ps_{n4}_{i}")
                                for k in range(KP):
                                    _mm(
                                        nc,
                                        ps[:],
                                        hT[:, 2 * k : 2 * k + 2, i * P : (i + 1) * P],
                                        wv[:, k, :, :],
                                        k == 0,
                                        k == KP - 1,
                                        pm=DR,
                                    )
                                nc.vector.scalar_tensor_tensor(
                                    v_aug[:, i, n4 * 4 : (n4 + 1) * 4, 0:D],
                                    ps[:].rearrange("p (h d) -> p h d", d=D),
                                    DQ,
                                    bv_bc[:, n4 * 288 : (n4 + 1) * 288].rearrange(
                                        "p (h d) -> p h d", d=D
                                    ),
                                    ALU.mult,
                                    ALU.add,
                                )

                # ======== phase C: q^T,k^T feature-major (fp8 DR) -> DRAM ========
                # ada (mlp half) interleaved so its W stream + GEMVs overlap
                # the qk matmuls instead of delaying attention start
                with (
                    tc.tile_pool(name=f"wqk_p{_rep}", bufs=2) as wqk_p,
                    tc.tile_pool(name=f"qkb_p{_rep}", bufs=2) as qkb_p,
                    tc.tile_pool(name=f"rows2b{_rep}", bufs=2) as tc_rows,
                    tc.tile_pool(name=f"wada2{_rep}", bufs=2) as wada2_p,
                    tc.tile_pool(name=f"qk_ps{_rep}", bufs=2, space="PSUM") as qk_ps,
                    tc.tile_pool(name=f"ada_ps2{_rep}", bufs=2, space="PSUM") as ada_ps2,
                ):
                    wqk_tiles = {}
                    for jb in range(2 * CT):
                        if jb % 2 == 0 and jb // 2 < 9:
                            ada_chunks(ada_ps2, wada2_p, tc_rows, 5 + jb // 2, 6 + jb // 2)
                        g, r = divmod(jb * P, 512)
                        if r == 0:
                            gw = min(512, 2 * C - g * 512)
                            gt = wqk_p.tile([P, KP, 2, 512], FP8, tag="wqk", name=f"wqkg_{g}")
                            nc.sync.dma_start(
                                gt[:, :, :, :gw], w_qkv[:, :, :, g * 512 : g * 512 + gw]
                            )
                            wqk_tiles[g] = gt
                        wt = wqk_tiles[g]
                        ps = qk_ps.tile([P, L], F32, tag="qkps", name=f"qk_ps_{jb}")
                        for k in range(KP):
                            for n2 in range(2):
                                _mm(
                                    nc,
                                    ps[:, n2 * 512 : (n2 + 1) * 512],
                                    wt[:, k, :, r : r + P],
                                    hT[:, 2 * k : 2 * k + 2, n2 * 512 : (n2 + 1) * 512],
                                    k == 0,
                                    k == KP - 1,
                                    pm=DR,
                                )
                        qkb = qkb_p.tile([P, L], BF16, tag="qkb", name=f"qkb_{jb}")
                        nc.scalar.activation(
                            qkb[:],
                            ps[:],
                            AF.Identity,
                            bias=bqkv_fm[:, jb : jb + 1],
                            scale=DQ,
                        )
                        nc.sync.dma_start(qkT_hbm[jb * P : (jb + 1) * P, :], qkb[:])

                # hoist the first head's q/k loads ahead of the ada-mlp stream
                qk0 = [const.tile([D, L], BF16, tag=f"qk0_{t}", name=f"qk0_{t}") for t in range(4)]
                for h in (0, 1):
                    nc.sync.dma_start(qk0[2 * h][:], qkT_hbm[h * D : (h + 1) * D, :])
                    nc.sync.dma_start(qk0[2 * h + 1][:], qkT_hbm[C + h * D : C + (h + 1) * D, :])

                # ======== remaining adaT ========
                nc.sync.dma_start(
                    adaT[:, 20:54], ada_hbm[0, 2560 : 6 * C].rearrange("(g p) -> p g", p=P)
                )
                nc.vector.tensor_scalar_add(sm1_mlp[:], adaT[:, 36:45], 1.0)

                # ======== phase D: attention, head at a time ========
                with (
                    tc.tile_pool(name=f"qkh{_rep}", bufs=4) as qkh,
                    tc.tile_pool(name=f"eTp{_rep}", bufs=3) as eTp,
                    tc.tile_pool(name=f"zp{_rep}", bufs=4) as zp,
                    tc.tile_pool(name=f"op{_rep}", bufs=2) as op_pool,
                    tc.tile_pool(name=f"s_ps{_rep}", bufs=2, space="PSUM") as s_ps,
                    tc.tile_pool(name=f"o_ps{_rep}", bufs=2, space="PSUM") as o_ps,
                ):
                    pend = []

                    def attn_finish():
                        h, o_sb, rz = pend.pop()
                        # broadcast 1/z to D partitions (PE) and scale o on DVE
                        psr = o_ps.tile([D, L], F32, tag="ops", name=f"psr_{h}")
                        for n2 in range(2):
                            sl = slice(n2 * 512, (n2 + 1) * 512)
                            _mm(nc, psr[:, sl], ones[:, 0:D], rz[:, sl], True, True)
                        oT = op_pool.tile([D, L], FP8, tag="oT", name=f"oT_{h}")
                        nc.vector.tensor_mul(oT[:], o_sb[0:D, :], psr[:])
                        nc.sync.dma_start(oT_hbm[h * D : (h + 1) * D, :], oT[:])

                    for h in range(H):
                        if h < 2:
                            qT, kT = qk0[2 * h][:], qk0[2 * h + 1][:]
                        else:
                            qTt = qkh.tile([D, L], BF16, tag="qkh", name=f"qT_{h}")
                            nc.sync.dma_start(qTt[:], qkT_hbm[h * D : (h + 1) * D, :])
                            kTt = qkh.tile([D, L], BF16, tag="qkh", name=f"kT_{h}")
                            nc.sync.dma_start(kTt[:], qkT_hbm[C + h * D : C + (h + 1) * D, :])
                            qT, kT = qTt[:], kTt[:]
                        pso = o_ps.tile([D + 1, L], F32, tag="ops", name=f"o_ps_{h}")
                        for m in range(LT):
                            sps = s_ps.tile([P, L], F32, tag="sps", name=f"s_ps_{h}_{m}")
                            for n2 in range(2):
                                _mm(
                                    nc,
                                    sps[:, n2 * 512 : (n2 + 1) * 512],
                                    kT[:, m * P : (m + 1) * P],
                                    qT[:, n2 * 512 : (n2 + 1) * 512],
                                    True,
                                    True,
                                )
                            eT = eTp.tile([P, L], BF16, tag="eT", name=f"eT_{h}_{m}")
                            nc.scalar.activation(eT[:], sps[:], AF.Exp, scale=SCL)
                            for n2 in range(2):
                                _mm(
                                    nc,
                                    pso[:, n2 * 512 : (n2 + 1) * 512],
                                    v_aug[:, m, h, :],
                                    eT[:, n2 * 512 : (n2 + 1) * 512],
                                    m == 0,
                                    m == LT - 1,
                                )
                            if m == 2 and pend:
                                attn_finish()
                        # drain o + softmax denominator (DVE only)
                        o_sb = zp.tile([D + 1, L], F32, tag="osb", name=f"osb_{h}", bufs=2)
                        nc.vector.tensor_copy(o_sb[:], pso[:])
                        z_row = zp.tile([1, L], F32, tag="z", name=f"z_{h}", bufs=4)
                        nc.sync.dma_start(z_row[:], o_sb[D : D + 1, :])
                        rz = zp.tile([1, L], F32R, tag="z", name=f"rz_{h}", bufs=4)
                        with nc.allow_low_precision(reason="f32r is fp32-width"):
                            nc.vector.reciprocal(rz[:], z_row[:].bitcast(F32R))
                        pend.append((h, o_sb, rz[:]))
                    attn_finish()

                # ======== phase E: proj (fp8 DR) + gated residual; LN2 folded ========
                oTb = hTs.tile([P, KB, L], FP8, tag="hT", name="oTb")
                nc.gpsimd.memset(oTb[:, KB - 1, :], 0.0)
                nc.sync.dma_start(
                    oTb[:, 0:CT, :], oT_hbm[:].rearrange("(k p) l -> p k l", p=P)
                )
                wpj = wq.tile([P, KP, 2, C], FP8, tag="wpj", name="wpj", bufs=1)
                nc.sync.dma_start(wpj[:], w_proj[:])
                h2T = hTs.tile([P, KB, L], FP8, tag="hT", name="h2T")
                nc.gpsimd.memset(h2T[:, KB - 1, :], 0.0)
                xh2s = []
                with (
                    tc.tile_pool(name=f"rows3{_rep}", bufs=2) as tc_rows,
                    tc.tile_pool(name=f"resp{_rep}", bufs=3) as resp,
                    tc.tile_pool(name=f"bc_ps3{_rep}", bufs=2, space="PSUM") as bc_ps3,
                    tc.tile_pool(name=f"pj_ps{_rep}", bufs=4, space="PSUM") as pj_ps,
                    tc.tile_pool(name=f"tr_ps2{_rep}", bufs=2, space="PSUM") as tr_ps2,
                ):
                    gmsa_bc = bcp.tile([P, C], F32, tag="bc", name="gmsa_bc")
                    bcast(tc, bc_ps3, gmsa_bc, ada_hbm[0:1, 2 * C : 3 * C], "gmsa")
                    bproj_bc = bcp.tile([P, C], F32, tag="bc", name="bproj_bc")
                    bcast(tc, bc_ps3, bproj_bc, b_proj.rearrange("(o n) -> o n", o=1), "bproj")
                    for i in range(LT):
                        for c0, cw in ((0, 512), (512, 512), (1024, 128)):
                            ps = pj_ps.tile([P, 512], F32, tag="pjps", name=f"pj_ps_{i}_{c0}")
                            for k in range(KP):
                                _mm(
                                    nc,
                                    ps[:, :cw],
                                    oTb[:, 2 * k : 2 * k + 2, i * P : (i + 1) * P],
                                    wpj[:, k, :, c0 : c0 + cw],
                                    k == 0,
                                    k == KP - 1,
                                    pm=DR,
                                )
                            sl = slice(c0, c0 + cw)
                            t = resp.tile([P, 512], F32, tag="res", name=f"res_{i}_{c0}")
                            nc.vector.scalar_tensor_tensor(
                                t[:, :cw], ps[:, :cw], DQ, bproj_bc[:, sl], ALU.mult, ALU.add
                            )
                            nc.vector.tensor_mul(t[:, :cw], t[:, :cw], gmsa_bc[:, sl])
                            nc.vector.tensor_add(x_sb[:, i, sl], x_sb[:, i, sl], t[:, :cw])
                        # LN2 stats/xhat for tile i overlap remaining proj work
                        xh = xhp.tile([P, C], BF16, tag="xh", name=f"xh2_{i}")
                        layernorm_xhat(i, xh)
                        xh2s.append(xh)
                        if i % 4 == 3:
                            modulate_transpose4(tr_ps2, i - 3, xh2s, h2T, 27, sm1_mlp)
                            xh2s = []

                # ======== phase G: MLP (fp8 DR) + final residual ========
                y_sb = pvp.tile([P, LT, C], F32, tag="pv", name="y_sb")
                JPC = FT // NCH  # 12 ff blocks per chunk
                with (
                    tc.tile_pool(name=f"rows4{_rep}", bufs=2) as tc_rows,
                    tc.tile_pool(name=f"wf1_p{_rep}", bufs=3) as wf1_p,
                    tc.tile_pool(name=f"uTs{_rep}", bufs=2) as uTs,
                    tc.tile_pool(name=f"resp2{_rep}", bufs=2) as resp2,
                ):
                    with tc.tile_pool(name=f"bc_ps4{_rep}", bufs=2, space="PSUM") as bc_ps4:
                        bfc2_bc = bcp.tile([P, C], F32, tag="bc", name="bfc2_bc")
                        bcast(tc, bc_ps4, bfc2_bc, b_fc2.rearrange("(o n) -> o n", o=1), "bfc2")
                        gmlp_bc = bcp.tile([P, C], F32, tag="bc", name="gmlp_bc")
                        bcast(tc, bc_ps4, gmlp_bc, ada_hbm[0:1, 5 * C : 6 * C], "gmlp")
                    mlp_ps = ExitStack()
                    f1_ps = mlp_ps.enter_context(tc.tile_pool(name=f"f1_ps{_rep}", bufs=2, space="PSUM"))
                    f2_ps = mlp_ps.enter_context(tc.tile_pool(name=f"f2_ps{_rep}", bufs=4, space="PSUM"))
                    for ch in range(NCH):
                        uT = uTs.tile([P, JPC, L], FP8, tag="uT", name=f"uT_{ch}")
                        for jj in range(JPC):
                            j = ch * JPC + jj
                            if j % 2 == 0:
                                wt4 = wf1_p.tile([P, KP, 2, 2 * P], FP8, tag="wf1", name=f"wfc1_{j}")
                                nc.sync.dma_start(wt4[:], w_fc1[:, :, :, j * P : (j + 2) * P])
                            jw = (j % 2) * P
                            ps = f1_ps.tile([P, L], F32, tag="f1ps", name=f"f1_ps_{j}")
                            for k in range(KP):
                                for n2 in range(2):
                                    _mm(
                                        nc,
                                        ps[:, n2 * 512 : (n2 + 1) * 512],
                                        wt4[:, k, :, jw : jw + P],
                                        h2T[:, 2 * k : 2 * k + 2, n2 * 512 : (n2 + 1) * 512],
                                        k == 0,
                                        k == KP - 1,
                                        pm=DR,
                                    )
                            # gelu(x) ~= silu(GF*x)/GF; the 1/GF lives in W_fc2,
                            # the GF in the prescaled b_fc1; DQ dequants the psum
                            nc.scalar.activation(
                                uT[:, jj, :],
                                ps[:],
                                AF.Silu,
                                bias=bfc1_fm[:, j : j + 1],
                                scale=GF * DQ,
                            )
                        for c0, cw in ((0, 512), (512, 512), (1024, 128)):
                            wf2 = wq.tile([P, JPC // 2, 2, 512], FP8, tag="wv", name=f"wfc2_{ch}_{c0}")
                            nc.sync.dma_start(
                                wf2[:, :, :, :cw],
                                w_fc2[:, ch * (JPC // 2) : (ch + 1) * (JPC // 2), :, c0 : c0 + cw],
                            )
                            sl = slice(c0, c0 + cw)
                            for i in range(LT):
                                ps = f2_ps.tile([P, 512], F32, tag="f2ps", name=f"f2_ps_{ch}_{c0}_{i}")
                                for kk in range(JPC // 2):
                                    _mm(
                                        nc,
                                        ps[:, :cw],
                                        uT[:, 2 * kk : 2 * kk + 2, i * P : (i + 1) * P],
                                        wf2[:, kk, :, :cw],
                                        kk == 0,
                                        kk == JPC // 2 - 1,
                                        pm=DR,
                                    )
                                if ch == 0:
                                    nc.vector.scalar_tensor_tensor(
                                        y_sb[:, i, sl], ps[:, :cw], DQ, bfc2_bc[:, sl],
                                        ALU.mult, ALU.add,
                                    )
                                elif ch < NCH - 1:
                                    nc.vector.scalar_tensor_tensor(
                                        y_sb[:, i, sl], ps[:, :cw], DQ, y_sb[:, i, sl],
                                        ALU.mult, ALU.add,
                                    )
                                else:
                                    t = resp2.tile([P, 512], F32, tag="res2", name=f"fres_{i}_{c0}")
                                    nc.vector.scalar_tensor_tensor(
                                        t[:, :cw], ps[:, :cw], DQ, y_sb[:, i, sl],
                                        ALU.mult, ALU.add,
                                    )
                                    nc.vector.tensor_mul(t[:, :cw], t[:, :cw], gmlp_bc[:, sl])
                                    nc.vector.tensor_add(
                                        x_sb[:, i, sl], x_sb[:, i, sl], t[:, :cw]
                                    )
                                    if c0 == 1024:
                                        nc.sync.dma_start(out_r[i], x_sb[:, i, :])

                    mlp_ps.close()

    nc.compile()
    return nc


_NC_CACHE = {}


def get_nc():
    if "nc" not in _NC_CACHE:
        _NC_CACHE["nc"] = build_nc()
    return _NC_CACHE["nc"]


def _pack_w(W, scale=SW):
    """[Cin, N] fp32 -> [128, ceil(Cin/256), 2, N] e4m3 with zero row pad."""
    fp8 = mybir.dt.np(FP8)
    Cin, N = W.shape
    kp = -(-Cin // 256)
    Wp = np.zeros((kp * 256, N), np.float32)
    Wp[:Cin] = np.clip(W * scale, -240.0, 240.0)
    return np.ascontiguousarray(
        Wp.reshape(kp, 2, P, N).transpose(2, 0, 1, 3).astype(fp8)
    )


def make_in_maps(inputs):
    import ml_dtypes

    B = inputs["x"].shape[0]
    f32 = lambda k: np.asarray(inputs[k], dtype=np.float32)
    shared = {
        "W_qkv": _pack_w(f32("W_qkv")),
        "W_proj": _pack_w(f32("W_proj")),
        "W_fc1": _pack_w(f32("W_fc1")),
        "W_fc2": _pack_w(f32("W_fc2") / GF),
        "W_ada": np.ascontiguousarray(f32("W_ada").astype(ml_dtypes.bfloat16)),
        "b_qkv": np.ascontiguousarray(f32("b_qkv")),
        "b_proj": np.ascontiguousarray(f32("b_proj")),
        "b_fc1": np.ascontiguousarray(f32("b_fc1") * GF),
        "b_fc2": np.ascontiguousarray(f32("b_fc2")),
        "b_ada": np.ascontiguousarray(f32("b_ada")),
    }
    in_maps = []
    for i in range(B):
        m = dict(shared)
        m["x"] = np.ascontiguousarray(np.asarray(inputs["x"][i], dtype=np.float32))
        m["c"] = np.ascontiguousarray(
            np.asarray(inputs["c"][i], dtype=np.float32).reshape(C)
        )
        in_maps.append(m)
    return in_maps


def kernel(**inputs):
    nc = get_nc()
    in_maps = make_in_maps(inputs)
    res = run_bass_kernel_spmd(nc, in_maps, list(range(len(in_maps))))
    return np.stack([r["out"] for r in res.results]).astype(np.float32)
